# revision 23
# baseline (speedup 1.0000x reference)
"""Causal self-attention (B=4, T=2048, C=1024, H=16, rope) on 8 trn2 cores.

Sharding: data-parallel over B (4) x tensor-parallel over heads (2 groups of
8 heads). Core (b, g) computes its batch's Q/K/V for its 8 heads, the full
causal attention for those heads, and a partial output projection
(y_heads @ wp_cols.T). Host sums the two head-group partials per batch and
adds the output bias.

v2 schedule notes (all per-core):
  - Inputs are loaded as fine-grained per-chunk tiles ordered by first use so
    the first projection matmul starts ~1-2us in instead of waiting ~45us for
    all loads.
  - Phase 1 PSUM->SBUF copies run on the scalar engine (idle in phase 1);
    rope muls are split gpsimd/vector so no engine queue lags the PE.
  - Phase 2 is software-pipelined: scores for step tj+1 are emitted before
    the PV matmuls of step tj, so the scalar-engine EXP never head-of-line
    blocks the in-order PE queue. Deferred epilogue/out-projection micro-ops
    are popped one per step to fill the remaining PE wait slots.
  - PSUM budget (8 banks): scores 2x[128,1024] (4) + O accum 3x[128,512] (3)
    + out-proj 1x[128,512] (1).
  - Softmax denominators come free as an all-ones 65th column of V; the
    normalization uses reciprocal_approx_fast (plenty for 2e-2 gate).
"""

import sys

if "/opt/trn_rl_repo" not in sys.path:
    sys.path.insert(0, "/opt/trn_rl_repo")

from collections import deque
from contextlib import ExitStack

import numpy as np

import concourse.bass as bass
import concourse.mybir as mybir
from concourse import bacc
from concourse.bass_utils import run_bass_kernel_spmd
from concourse.tile import TileContext

B, T, C = 4, 2048, 1024
H = 16
D = 64
NCORES = 8
CL = C // 2  # per-core c_out (8 heads * 64)
HL = 8  # local heads
F = mybir.dt.float32
FR = mybir.dt.bfloat16  # matmul operand dtype

_NC_CACHE = {}


def _build_nc(with_bias: bool):
    KC = 9 if with_bias else 8  # c_in chunks of 128 (one extra for bias row)
    CIN = KC * 128
    nc = bacc.Bacc("TRN2", debug=False, num_devices=NCORES)

    xT = nc.declare_dram_parameter("xT", [CIN, T], FR, isOutput=False).ap()
    wqT = nc.declare_dram_parameter("wqT", [CIN, CL], FR, isOutput=False).ap()
    wkT = nc.declare_dram_parameter("wkT", [CIN, CL], FR, isOutput=False).ap()
    wvT = nc.declare_dram_parameter("wvT", [CIN, CL], FR, isOutput=False).ap()
    wpT = nc.declare_dram_parameter("wpT", [CL, C], FR, isOutput=False).ap()
    ones8 = nc.declare_dram_parameter("ones8", [128, HL], FR, isOutput=False).ap()
    ropeC = nc.declare_dram_parameter("ropeC", [128, T], FR, isOutput=False).ap()
    ropeS = nc.declare_dram_parameter("ropeS", [128, T], FR, isOutput=False).ap()
    dmask = nc.declare_dram_parameter("dmask", [128, 128], FR, isOutput=False).ap()
    selm = nc.declare_dram_parameter("selm", [64, 256], FR, isOutput=False).ap()
    selD = nc.declare_dram_parameter("selD", [2, 128], F, isOutput=False).ap()
    out = nc.declare_dram_parameter("out", [T, C], F, isOutput=True).ap()

    EXP = mybir.ActivationFunctionType.Exp
    scale = 1.0 / float(np.sqrt(D))

    with TileContext(nc) as tc:
        with ExitStack() as ctx:
            # pools that live across both phases
            qk_pool = ctx.enter_context(tc.tile_pool(name="qk", bufs=1))
            v_pool = ctx.enter_context(tc.tile_pool(name="v", bufs=1))
            c2 = ctx.enter_context(tc.tile_pool(name="c2", bufs=1))

            qt_sb = [
                qk_pool.tile([128, T], FR, tag=f"qt{m}", name=f"qt{m}")
                for m in range(4)
            ]
            kt_sb = [
                qk_pool.tile([128, T], FR, tag=f"kt{m}", name=f"kt{m}")
                for m in range(4)
            ]
            vaug = [
                v_pool.tile([128, HL, D + 1], FR, tag=f"va{j}", name=f"va{j}")
                for j in range(16)
            ]
            wp_sb = c2.tile([128, 4, C], FR, tag="wp", name="wp")
            dm2 = c2.tile([128, 2, 128], FR, tag="dm", name="dm")
            selm_sb = c2.tile([64, 256], FR, tag="selm", name="selm")
            selD_sb = c2.tile([2, 128], F, tag="selD", name="selD")

            # ---------------- phase-2 pools (created early: ci=0 attention is
            # interleaved into phase 1, so its PSUM/SBUF must coexist with ps1)
            ppool = ctx.enter_context(tc.tile_pool(name="pt", bufs=3))
            yrawp = ctx.enter_context(tc.tile_pool(name="yraw", bufs=3))
            ytmpp = ctx.enter_context(tc.tile_pool(name="ytmp", bufs=3))
            ynp = ctx.enter_context(tc.tile_pool(name="yn", bufs=9))
            dpool = ctx.enter_context(tc.tile_pool(name="dd", bufs=2))
            d16p = ctx.enter_context(tc.tile_pool(name="d16", bufs=3))
            bcsp = ctx.enter_context(tc.tile_pool(name="bcs", bufs=2))
            spool = ctx.enter_context(tc.tile_pool(name="sps", bufs=2, space="PSUM"))
            opool = ctx.enter_context(tc.tile_pool(name="ops", bufs=2, space="PSUM"))

            pending = deque()

            def pop_pieces(n):
                k = n if len(pending) > 24 else min(n, 1)
                for _ in range(min(k, len(pending))):
                    pending.popleft()()

            fin_i = [0]

            def push_epilogue(yraw, ytmp, yn_list):
                d16 = d16p.tile([128, 512], FR, tag="D16", name="D16")
                d_sb = dpool.tile([128, 1024], F, tag="D", name="D")

                def e3():
                    nc.sync.dma_start(out=d16[0:1, :], in_=yraw[64:65, :])
                    nc.sync.dma_start(out=d16[1:2, :], in_=ytmp[64:65, :])
                    nc.vector.tensor_copy(d_sb[0:2, 0:512], d16[0:2, :])

                def e4():
                    nc.vector.reciprocal_approx_fast(
                        out=d_sb[0:2, 512:1024], in_=d_sb[0:2, 0:512]
                    )

                def e5():
                    mb = spool.tile([128, 1024], F, tag="s", name="mb")
                    m_ps = mb[:, 0:512]
                    bc_ps = mb[:, 512:1024]
                    nc.tensor.matmul(
                        m_ps,
                        lhsT=selm_sb[:, 0:128],
                        rhs=yraw[0:64, :],
                        start=True,
                        stop=False,
                        skip_group_check=True,
                    )
                    nc.tensor.matmul(
                        m_ps,
                        lhsT=selm_sb[:, 128:256],
                        rhs=ytmp[0:64, :],
                        start=False,
                        stop=True,
                        skip_group_check=True,
                    )
                    nc.tensor.matmul(
                        bc_ps,
                        lhsT=selD_sb,
                        rhs=d_sb[0:2, 512:1024],
                        start=True,
                        stop=True,
                        skip_group_check=True,
                    )
                    bcs = bcsp.tile([128, 512], F, tag="bcs", name="bcs")
                    nc.vector.tensor_copy(bcs, bc_ps)
                    ynorm = ynp.tile([128, 512], FR, tag="yn", name="yn")
                    nc.vector.tensor_mul(ynorm, m_ps, bcs)
                    yn_list.append(ynorm)

                pending.extend((e3, e4, e5))

            def attn_block(ci, p, yn_list):
                ntj = 4 * ci + 4
                o_ps = [
                    opool.tile([128, 512], F, tag="o", name="o") for _ in range(2)
                ]
                s_tiles = [None] * ntj

                def emit_S(tj):
                    kk = tj - 4 * ci
                    off = 128 * max(kk, 0)
                    s_ps = spool.tile([128, 1024], F, tag="s", name="s")
                    s_tiles[tj] = s_ps
                    for h in range(2):
                        nc.tensor.matmul(
                            s_ps[:, 512 * h + off : 512 * h + 512],
                            lhsT=kt_sb[p][
                                64 * h : 64 * h + 64,
                                128 * tj : 128 * (tj + 1),
                            ],
                            rhs=qt_sb[p][
                                64 * h : 64 * h + 64,
                                512 * ci + off : 512 * (ci + 1),
                            ],
                            start=True,
                            stop=True,
                            tile_position=(64 * h, 0),
                        )

                def emit_exp(tj):
                    kk = tj - 4 * ci
                    off = 128 * max(kk, 0)
                    s_ps = s_tiles[tj]
                    pt = ppool.tile([128, 1024], FR, tag="pt", name="pt")
                    if kk < 0:
                        nc.scalar.activation(pt, s_ps, EXP, scale=scale)
                    else:
                        s_v = s_ps.rearrange("q (h n) -> q h n", h=2)[:, :, off:]
                        p_v = pt.rearrange("q (h n) -> q h n", h=2)[:, :, off:]
                        nc.scalar.activation(p_v, s_v, EXP, scale=scale)
                        blk = pt.rearrange("q (h n) -> q h n", h=2)[
                            :, :, off : off + 128
                        ]
                        nc.gpsimd.tensor_mul(blk, blk, dm2)
                    return pt

                def emit_P(tj, pt):
                    kk = tj - 4 * ci
                    off = 128 * max(kk, 0)
                    for h in range(2):
                        nc.tensor.matmul(
                            o_ps[h][0 : D + 1, off:512],
                            lhsT=vaug[tj][:, 2 * p + h, :],
                            rhs=pt[:, 512 * h + off : 512 * h + 512],
                            start=(tj == 0),
                            stop=(tj == ntj - 1),
                            skip_group_check=True,
                        )

                emit_S(0)
                for tj in range(ntj):
                    if tj + 1 < ntj:
                        emit_S(tj + 1)
                    pt = emit_exp(tj)
                    pop_pieces(3 if ci == 3 else 2)
                    emit_P(tj, pt)
                # drain O PSUM right away (bf16; row 64 keeps denominators)
                yraw = yrawp.tile([128, 512], FR, tag="yraw", name="yraw")
                ytmp = ytmpp.tile([128, 512], FR, tag="ytmp", name="ytmp")
                nc.vector.tensor_copy(yraw[0:65, :], o_ps[0][0:65, :])
                nc.vector.tensor_copy(ytmp[0:65, :], o_ps[1][0:65, :])
                push_epilogue(yraw, ytmp, yn_list)

            yn_ci0 = []

            # ---------------- phase 1: QKV projections + rope ----------------
            with ExitStack() as p1:
                wpool = p1.enter_context(tc.tile_pool(name="w", bufs=1))
                xpool = p1.enter_context(tc.tile_pool(name="x", bufs=1))
                rpool = p1.enter_context(tc.tile_pool(name="rope", bufs=1))
                tpool = p1.enter_context(tc.tile_pool(name="t1", bufs=2))
                ps1 = p1.enter_context(tc.tile_pool(name="ps1", bufs=2, space="PSUM"))

                wv_k = [
                    wpool.tile([128, CL], FR, tag=f"wv{k}", name=f"wv{k}")
                    for k in range(KC)
                ]
                wq_k = [
                    wpool.tile([128, CL], FR, tag=f"wq{k}", name=f"wq{k}")
                    for k in range(KC)
                ]
                wk_k = [
                    wpool.tile([128, CL], FR, tag=f"wk{k}", name=f"wk{k}")
                    for k in range(KC)
                ]
                xc = [
                    [
                        xpool.tile([128, 512], FR, tag=f"x{t}_{k}", name=f"x{t}_{k}")
                        for k in range(KC)
                    ]
                    for t in range(4)
                ]
                rcF = rpool.tile([128, T], FR, tag="rcF", name="rcF")
                rsF = rpool.tile([128, T], FR, tag="rsF", name="rsF")

                # loads: split across the three DMA-capable sequencers so
                # descriptor generation parallelizes; ordered by first use
                for k in range(KC):
                    nc.sync.dma_start(out=wv_k[k], in_=wvT[128 * k : 128 * (k + 1), :])
                    nc.sync.dma_start(
                        out=xc[0][k], in_=xT[128 * k : 128 * (k + 1), 0:512]
                    )
                for k in range(KC):
                    nc.scalar.dma_start(
                        out=xc[1][k],
                        in_=xT[128 * k : 128 * (k + 1), 512 : 1024],
                    )
                    nc.gpsimd.dma_start(
                        out=xc[2][k],
                        in_=xT[128 * k : 128 * (k + 1), 1024 : 1536],
                    )
                for k in range(KC):
                    nc.sync.dma_start(
                        out=xc[3][k],
                        in_=xT[128 * k : 128 * (k + 1), 1536 : 2048],
                    )
                for k in range(KC):
                    nc.scalar.dma_start(
                        out=wq_k[k], in_=wqT[128 * k : 128 * (k + 1), :]
                    )
                    nc.gpsimd.dma_start(
                        out=wk_k[k], in_=wkT[128 * k : 128 * (k + 1), :]
                    )
                nc.sync.dma_start(out=rcF, in_=ropeC)
                nc.sync.dma_start(out=rsF, in_=ropeS)
                for p in range(4):
                    nc.scalar.dma_start(
                        out=wp_sb[:, p, :], in_=wpT[128 * p : 128 * (p + 1), :]
                    )
                for j in range(16):
                    nc.gpsimd.dma_start(out=vaug[j][:, :, D : D + 1], in_=ones8)
                for h in range(2):
                    nc.gpsimd.dma_start(out=dm2[:, h, :], in_=dmask)
                nc.gpsimd.dma_start(out=selm_sb, in_=selm)
                nc.gpsimd.dma_start(out=selD_sb, in_=selD)

                # V tiles (natural [t, c_out] layout) -> vaug
                for jj in range(16):
                    t, tt = jj // 4, jj % 4
                    ps = ps1.tile([128, 512], F, tag="ps", name="ps")
                    for k in range(KC):
                        nc.tensor.matmul(
                            ps,
                            lhsT=xc[t][k][:, 128 * tt : 128 * (tt + 1)],
                            rhs=wv_k[k],
                            start=(k == 0),
                            stop=(k == KC - 1),
                        )
                    nc.vector.tensor_copy(
                        out=vaug[jj][:, :, 0:D],
                        in_=ps.rearrange("p (h d) -> p h d", h=HL),
                    )

                # Q^T / K^T strips ([c_out, t] layout) + rope, pair-major.
                # Rope per [128, T] strip: one partition-block-swapped copy
                # (4 DMAs of [32, T]), then q*C + swap(q)*S' muls.
                # After each pair, the ci=0 attention for that pair is
                # emitted: the PE chews it while the next pair's rope runs.
                strip_i = 0
                for m in range(4):
                    for wk_, dst in ((wk_k, kt_sb), (wq_k, qt_sb)):
                        qcp = tpool.tile([128, T], FR, tag="qcp", name="qcp")
                        for t in range(4):
                            ps = ps1.tile([128, 512], F, tag="ps", name="ps")
                            for k in range(KC):
                                nc.tensor.matmul(
                                    ps,
                                    lhsT=wk_[k][:, 128 * m : 128 * (m + 1)],
                                    rhs=xc[t][k],
                                    start=(k == 0),
                                    stop=(k == KC - 1),
                                )
                            nc.scalar.copy(qcp[:, 512 * t : 512 * (t + 1)], ps)
                        qsw = tpool.tile([128, T], FR, tag="qsw", name="qsw")
                        eng = nc.sync if strip_i % 2 == 0 else nc.gpsimd
                        for a, b in ((0, 32), (32, 0), (64, 96), (96, 64)):
                            eng.dma_start(
                                out=qsw[a : a + 32, :], in_=qcp[b : b + 32, :]
                            )
                        strip_i += 1
                        t1 = tpool.tile([128, T], FR, tag="t1", name="t1")
                        t2 = tpool.tile([128, T], FR, tag="t2", name="t2")
                        nc.vector.tensor_mul(t1, qcp, rcF)
                        nc.vector.tensor_mul(t2, qsw, rsF)
                        nc.vector.tensor_add(dst[m], t1, t2)
                    # attention for pair m-1 runs while pair m's rope drains
                    if m > 0:
                        attn_block(0, m - 1, yn_ci0)
                attn_block(0, 3, yn_ci0)

            # ---------------- phase 2: remaining attention + out-projection --
            osbp = ctx.enter_context(tc.tile_pool(name="osb", bufs=3))
            jpool = ctx.enter_context(tc.tile_pool(name="jps", bufs=2, space="PSUM"))

            def push_outproj(ci, yn_list):
                for tt in range(4):
                    osb = osbp.tile([128, 1024], F, tag="osb", name="osb")
                    for cc in range(2):
                        pr = jpool.tile([128, 512], F, tag="opj", name="opj")

                        def mk_mm(p, pr=pr, tt=tt, cc=cc):
                            def mm():
                                nc.tensor.matmul(
                                    pr,
                                    lhsT=yn_list[p][:, 128 * tt : 128 * (tt + 1)],
                                    rhs=wp_sb[:, p, 512 * cc : 512 * (cc + 1)],
                                    start=(p == 0),
                                    stop=(p == 3),
                                    skip_group_check=True,
                                )

                            return mm

                        for p in range(4):
                            pending.append(mk_mm(p))

                        def cpy(pr=pr, osb=osb, cc=cc):
                            nc.vector.tensor_copy(
                                osb[:, 512 * cc : 512 * (cc + 1)], pr
                            )

                        pending.append(cpy)

                    def fin(osb=osb, tt=tt):
                        eng = nc.sync if fin_i[0] % 2 == 0 else nc.gpsimd
                        fin_i[0] += 1
                        eng.dma_start(
                            out=out[
                                512 * ci + 128 * tt : 512 * ci + 128 * (tt + 1), :
                            ],
                            in_=osb,
                        )

                    pending.append(fin)

            push_outproj(0, yn_ci0)
            for ci in range(1, 4):
                yn_list = []
                for p in range(4):
                    attn_block(ci, p, yn_list)
                push_outproj(ci, yn_list)
            while pending:
                pending.popleft()()

    nc.compile()
    return nc


def _get_nc(with_bias: bool):
    if with_bias not in _NC_CACHE:
        _NC_CACHE[with_bias] = _build_nc(with_bias)
    return _NC_CACHE[with_bias]


def _rope_tables():
    half = D // 2
    i = np.arange(half, dtype=np.float32)
    expo = (2.0 * i / np.float32(D)).astype(np.float32)
    alpha = (1.0 / (np.float32(10000.0) ** expo)).astype(np.float32)
    ang = (np.arange(T, dtype=np.float32)[:, None] * alpha[None, :]).astype(np.float32)
    cosv = np.cos(ang).astype(np.float32).T  # [32, T]
    sinv = np.sin(ang).astype(np.float32).T
    c64 = np.concatenate([cosv, cosv], axis=0)  # [64, T]
    s64 = np.concatenate([-sinv, sinv], axis=0)
    ropeC = np.ascontiguousarray(np.concatenate([c64, c64], axis=0))  # [128, T]
    ropeS = np.ascontiguousarray(np.concatenate([s64, s64], axis=0))
    import ml_dtypes

    return ropeC.astype(ml_dtypes.bfloat16), ropeS.astype(ml_dtypes.bfloat16)


import ml_dtypes


def _round_fp32r(a):
    """Cast host data to the matmul operand dtype (bf16)."""
    return np.ascontiguousarray(np.asarray(a, dtype=np.float32).astype(ml_dtypes.bfloat16))


def _make_in_maps(x, wq, bq, wk, bk, wv, bv, wp, with_bias):
    ropeC, ropeS = _rope_tables()
    # multiplicative causal mask for the diagonal 128x128 block (j <= i keeps)
    dmask = np.triu(np.ones((128, 128), np.float32)).astype(ml_dtypes.bfloat16)
    ones8 = np.ones((128, HL), dtype=ml_dtypes.bfloat16)
    # partition-merge selector: [I64|0] then [0|I64] (stacks h0/h1 rows)
    selm = np.zeros((64, 256), np.float32)
    selm[:, 0:64] = np.eye(64)
    selm[:, 192:256] = np.eye(64)
    selm = np.ascontiguousarray(selm).astype(ml_dtypes.bfloat16)
    # denominator-broadcast selector: row h -> output partitions 64h..64h+63
    selD = np.zeros((2, 128), np.float32)
    selD[0, 0:64] = 1.0
    selD[1, 64:128] = 1.0
    selD = np.ascontiguousarray(selD)
    in_maps = []
    for b in range(B):
        xb = np.ascontiguousarray(x[b].T.astype(np.float32, copy=False))  # [C, T]
        if with_bias:
            aug = np.zeros((9 * 128 - C, T), np.float32)
            aug[0, :] = 1.0
            xb = np.concatenate([xb, aug], axis=0)
        for g in range(2):
            sl = slice(g * CL, (g + 1) * CL)
            wqTc = np.ascontiguousarray(wq[sl, :].T.astype(np.float32, copy=False))
            wkTc = np.ascontiguousarray(wk[sl, :].T.astype(np.float32, copy=False))
            wvTc = np.ascontiguousarray(wv[sl, :].T.astype(np.float32, copy=False))
            if with_bias:
                npad = 9 * 128 - C

                def _aug_w(wT, bias):
                    a = np.zeros((npad, CL), np.float32)
                    a[0, :] = bias[sl].astype(np.float32, copy=False)
                    return np.ascontiguousarray(np.concatenate([wT, a], axis=0))

                wqTc = _aug_w(wqTc, bq)
                wkTc = _aug_w(wkTc, bk)
                wvTc = _aug_w(wvTc, bv)
            wpTc = np.ascontiguousarray(wp[:, sl].T.astype(np.float32, copy=False))
            in_maps.append(
                {
                    "xT": _round_fp32r(xb),
                    "wqT": _round_fp32r(wqTc),
                    "wkT": _round_fp32r(wkTc),
                    "wvT": _round_fp32r(wvTc),
                    "wpT": _round_fp32r(wpTc),
                    "ones8": ones8,
                    "ropeC": ropeC,
                    "ropeS": ropeS,
                    "dmask": dmask,
                    "selm": selm,
                    "selD": selD,
                }
            )
    return in_maps


def _gather(results, bp):
    out = np.empty((B, T, C), dtype=np.float32)
    bp32 = np.asarray(bp, dtype=np.float32)
    for b in range(B):
        out[b] = results[2 * b]["out"] + results[2 * b + 1]["out"] + bp32
    return out


def run(x, wq, bq, wk, bk, wv, bv, wp, bp, trace=False, **kw):
    """Build/compile (cached), run on 8 cores, gather. Returns (out, results)."""
    arrs = [np.asarray(a) for a in (x, wq, bq, wk, bk, wv, bv, wp, bp)]
    x, wq, bq, wk, bk, wv, bv, wp, bp = arrs
    with_bias = bool(np.any(bq) or np.any(bk) or np.any(bv))
    nc = _get_nc(with_bias)
    in_maps = _make_in_maps(x, wq, bq, wk, bk, wv, bv, wp, with_bias)
    res = run_bass_kernel_spmd(nc, in_maps, list(range(NCORES)), trace=trace, **kw)
    return _gather(res.results, bp), res


def kernel(x, wq, bq, wk, bk, wv, bv, wp, bp):
    out, _ = run(x, wq, bq, wk, bk, wv, bv, wp, bp)
    return out
